# revision 56
# baseline (speedup 1.0000x reference)
"""Trainium2 Bass kernel for nn_BDH_1726576853700 (sparse_attention).

3-layer sparse-attention net: B=1, T=1024, D=256, NH=4, N=8192, VOCAB=256.

Sharding over 8 NeuronCores: device d -> (head h=d//2, half=d%2) — each device
owns a 4096-wide slice of one head's sparse latent dim.  Within the slice the
latent index is permuted evens-first so the RoPE pair partner sits exactly 2048
rows away (k-tile (0,p) <-> (1,p)), turning the pair rotation into whole-tile
elementwise ops.  Per layer:
  - x_sparse^T = relu(enc^T @ x^T)      (local, fp16)
  - qr = rope(x_sparse) -> quantize fp8 (DVE rope, Pool fp8 convert)
  - S_partial = qr^T qr via fp8 DoubleRow matmuls (two k-tiles contracted per
    instruction = 2x PE throughput); strictly-causal upper triangle only,
    diagonal blocks masked
  - ykv_partial = S^T @ x ; pair AllReduce (the two halves of one head)
  - ykv_ln = layernorm(ykv); y_sparse^T = relu(encv^T ykv_ln^T)   (fp16)
  - ymlp^T accumulated transposed (lhsT = decoder tile), transposed back to
    natural [T, D] before the 8-way AllReduce; x = ln(x + ln(ymlp))

Perf structure: phase A (enc proj) is emitted interleaved with rope and the
first S accumulation pass (causal rows 0..2 live in 6 PSUM banks, accumulated
pair-major so S consumes qr pairs as rope emits them, 2 pairs per rope op).
Rows 3..7 run chunk-major after rope completes, interleaved with the ykv
accumulation.  LayerNorms are batched: stats for all 8 t-tiles first, one
Act Rsqrt for all 8, then in-place applies — one cross-engine round trip
instead of eight.  Weights/tables are loaded in few large DMAs, prefetched a
phase ahead on the SP queue.  Collectives run in fp16; matmuls in fp16
(scores fp8) with fp32 PSUM accumulation.

PSUM discipline: concurrent accumulation groups never share a 2KB bank
(start=True clears has_written bits for the whole bank).
"""

import math
import sys

for _p in ("/opt/trn_rl_repo",):
    if _p not in sys.path:
        sys.path.insert(0, _p)

import numpy as np

import concourse.bass as bass
import concourse.mybir as mybir
import concourse.tile as tile
from concourse import bacc, bass_utils
from concourse.masks import make_identity

# ---- problem constants (hardcoded per contract) ----
B, T, D, NH, N = 1, 1024, 256, 4, 8192
VOCAB = 256
N_LAYER = 3
EPS = 1e-5
TWO_PI = 2.0 * math.pi
N_CORES = 8
NLOC = N // 2          # latent columns per device: 4096
P = 128
NT = T // P            # 8 t-tiles
KD = D // P            # 2 d-tiles
NM = NLOC // P         # 32 n-tiles per device
NPAIR = NM // 2        # 16 rope pairs
NG = NPAIR // 2        # 8 two-pair rope groups
HDT = mybir.dt.float16     # on-chip activation dtype
F8 = mybir.dt.float8e4     # scores operand dtype (TRN e4m3, max 240)
F32 = mybir.dt.float32
YKV_SCALE = 1.0 / 256.0    # keeps ykv in fp16 range; LN downstream is
                           # scale-invariant so the result is unchanged
DR = mybir.MatmulPerfMode.DoubleRow
RELU = mybir.ActivationFunctionType.Relu
RSQRT = mybir.ActivationFunctionType.Rsqrt

# compact S storage: row-tile i holds causal cols [128*i, 1024)
S_OFF = [0]
for _i in range(1, NT):
    S_OFF.append(S_OFF[-1] + (T - P * (_i - 1)))
S_TOT = S_OFF[-1] + (T - P * (NT - 1))   # 4608

# S accumulation chunks (row, col_base, width); pass A rows 0..2 are
# pair-major (6 one-bank accumulators beside psA's 2 banks); row 3 takes
# over psA's freed banks once the enc-proj is done (pass B0, still mostly
# pair-major); rows 4..7 run chunk-major after rope completes (pass B1).
PASSA = [(0, 0, 512), (0, 512, 512), (1, 128, 384), (1, 512, 512),
         (2, 256, 256), (2, 512, 512)]
PASSB0 = [(3, 384, 128), (3, 512, 512)]
PASSB1 = [(4, 512, 512), (5, 640, 384), (6, 768, 256), (7, 896, 128)]

_CACHE = {}


def _build_program(dbg=False, use_collectives=True, n_layers=N_LAYER):
    def emit_allreduce(nc, groups, ins, outs):
        if use_collectives:
            nc.gpsimd.collective_compute(
                "AllReduce", mybir.AluOpType.add, replica_groups=groups,
                ins=ins, outs=outs)
        else:
            # timing/sim variant: replace the collective with a plain copy
            nc.sync.dma_start(outs[0], ins[0])

    nc = bacc.Bacc("TRN2", target_bir_lowering=False, debug=False,
                   num_devices=N_CORES)

    x0_d = nc.dram_tensor("x0", [T, D], HDT, kind="ExternalInput")
    encw_d = nc.dram_tensor("encw", [D, NLOC], HDT, kind="ExternalInput")
    encvw_d = nc.dram_tensor("encvw", [D, NLOC], HDT, kind="ExternalInput")
    decw_d = nc.dram_tensor("decw", [NLOC, D], HDT, kind="ExternalInput")
    ct_d = nc.dram_tensor("ct", [NLOC // 2, T], HDT, kind="ExternalInput")
    st_d = nc.dram_tensor("st", [NLOC // 2, T], HDT, kind="ExternalInput")
    lmh_d = nc.dram_tensor("lmh", [D, VOCAB], HDT, kind="ExternalInput")
    umask_d = nc.dram_tensor("umask", [P, P], F32, kind="ExternalInput")
    logits_d = nc.dram_tensor("logits", [T, VOCAB], F32, kind="ExternalOutput")

    PAIR_GROUPS = [[0, 1], [2, 3], [4, 5], [6, 7]]
    ALL_GROUP = [list(range(N_CORES))]

    with tile.TileContext(nc) as tc:
        persist = tc.alloc_tile_pool(name="persist", bufs=1)
        dram = tc.alloc_tile_pool(name="dram", bufs=1, space="DRAM")

        # persistent SBUF state
        x_h = persist.tile([P, NT, D], HDT)         # residual (natural)
        xT_h = persist.tile([P, KD, T], HDT)        # x^T fp16
        ykvT_h = persist.tile([P, KD, T], HDT)      # ykv_ln^T fp16
        x_sp = persist.tile([P, 2, NPAIR, T], HDT)  # x_sparse^T tiles
        qr8 = persist.tile([P, 2, NPAIR, T], F8)    # roped x_sparse^T, fp8
        s16 = persist.tile([P, S_TOT], HDT)         # causal S rows, compact
        umask_sb = persist.tile([P, P], F32)
        ident = persist.tile([P, P], HDT)
        eps_sb = persist.tile([P, 1], F32)

        # weights: one big DMA each, single-buffered (cross-layer WAR
        # rotation handled by the tile framework)
        encp = tc.alloc_tile_pool(name="encp", bufs=1)
        encvp = tc.alloc_tile_pool(name="encvp", bufs=1)
        decp = tc.alloc_tile_pool(name="decp", bufs=1)
        # rope tables: 2-pair chunks, double buffered per table
        ctp = tc.alloc_tile_pool(name="ctp", bufs=2)
        stp = tc.alloc_tile_pool(name="stp", bufs=2)
        # fp16 scratch (2 tags x bufs=2 x [P,2,T]): rope temps, ysp/xy,
        # ymlpT staging, logits staging
        wk16 = tc.alloc_tile_pool(name="wk16", bufs=2)
        # staging ring: x0 / ykv pre/post / ymlp nat pre/post share one
        # buffer, WAR-serialized through the per-layer dataflow
        stg = tc.alloc_tile_pool(name="stg", bufs=1)
        statp = tc.alloc_tile_pool(name="statp", bufs=2)

        nc.vector.memset(eps_sb[:], float(EPS))
        make_identity(nc, ident[:])

        def ln_batch(src, out_fn, lo=0, hi=NT):
            """Batched LayerNorm over tiles lo..hi of a [P, NT, D] tile
            `src`; out_fn(j) gives the output AP for tile j (may alias src
            for in-place).  Caller can process halves so consumers of the
            first tiles start before the second half's stats."""
            statsb = statp.tile([P, NT, 6], F32, name="ln_stats")
            mvb = statp.tile([P, NT, 2], F32, name="ln_mv")
            rstdb = statp.tile([P, NT], F32, name="ln_rstd")
            for j in range(lo, hi):
                nc.vector.bn_stats(out=statsb[:, j, :], in_=src[:, j, :])
                nc.vector.bn_aggr(out=mvb[:, j, :], in_=statsb[:, j, :])
            nc.scalar.activation(out=rstdb[:, lo:hi], in_=mvb[:, lo:hi, 1],
                                 func=mybir.ActivationFunctionType.Sqrt,
                                 bias=eps_sb[:])
            nc.vector.reciprocal(out=rstdb[:, lo:hi], in_=rstdb[:, lo:hi])
            for j in range(lo, hi):
                nc.vector.tensor_scalar(out=out_fn(j), in0=src[:, j, :],
                                        scalar1=mvb[:, j, 0:1],
                                        scalar2=rstdb[:, j:j + 1],
                                        op0=mybir.AluOpType.subtract,
                                        op1=mybir.AluOpType.mult)

        def transpose_into(dst_ap, src_ap, pst_pool, copy_eng=None):
            """PE-transpose a [P, P] fp16 SBUF block into dst (via PSUM)."""
            pst = pst_pool.tile([P, P], HDT, name="pst")
            nc.tensor.transpose(pst[:], src_ap, ident[:])
            if copy_eng is nc.scalar:
                nc.scalar.copy(out=dst_ap, in_=pst[:])
            else:
                nc.vector.tensor_copy(out=dst_ap, in_=pst[:])

        def gated(t, gate):
            """WAW-dummy: delay t's load until `gate` (tiny tile) is written,
            keeping long prefetch transfers off the AR critical window.
            Touches one element of every dim-1 slice so each partial-load DMA
            picks up the ordering dependency."""
            if gate is not None:
                nc.vector.tensor_scalar_mul(out=t[:, :, 0:1], in0=t[:, :, 0:1],
                                            scalar1=gate[:, 0:1])

        def load_enc(which, gate=None):
            """Load enc/encv [P, KD, NLOC] in two DMAs (8KB contig rows)."""
            d = encw_d if which == 0 else encvw_d
            pool = encp if which == 0 else encvp
            t = pool.tile([P, KD, NLOC], HDT, name=f"w{which}")
            gated(t, gate)
            src = d.ap().rearrange("(k p) n -> p k n", p=P)
            for k in range(KD):
                nc.sync.dma_start(t[:, k, :], src[:, k, :])
            return t

        def load_dec(gate=None):
            t = decp.tile([P, NM, D], HDT, name="dec")
            gated(t, gate)
            src = decw_d.ap().rearrange("(m p) d -> p m d", p=P)
            for h in range(2):
                nc.sync.dma_start(t[:, h * 16:(h + 1) * 16, :],
                                  src[:, h * 16:(h + 1) * 16, :])
            return t

        def load_tab(g, gate=None):
            """Load rope tables for 2-pair group g: [P, 2, T] each."""
            ctt = ctp.tile([P, 2, T], HDT, name="ct")
            stt = stp.tile([P, 2, T], HDT, name="st")
            gated(ctt, gate)
            gated(stt, gate)
            src_c = ct_d.ap().rearrange("(i p) t -> p i t", p=P)
            src_s = st_d.ap().rearrange("(i p) t -> p i t", p=P)
            nc.sync.dma_start(ctt[:], src_c[:, 2 * g:2 * g + 2, :])
            nc.sync.dma_start(stt[:], src_s[:, 2 * g:2 * g + 2, :])
            return ctt, stt

        def s16_store(ps, r, base, w, eng=0):
            """Copy a PSUM S chunk into compact fp16 storage (+ diag mask).
            eng picks the copy engine (0=Act, 1=DVE) so the burst of
            end-of-pass stores spreads across idle engines (GPSIMD cannot
            read PSUM)."""
            def cp(dst, src):
                if eng == 1:
                    nc.vector.tensor_copy(out=dst, in_=src)
                else:
                    nc.scalar.copy(out=dst, in_=src)
            off = S_OFF[r] + (base - r * P)
            if base == r * P:     # chunk starts at the diagonal block
                nc.vector.tensor_mul(s16[:, off:off + P],
                                     ps[:, 0:P], umask_sb[:])
                if w > P:
                    cp(s16[:, off + P:off + w], ps[:, P:w])
            else:
                cp(s16[:, off:off + w], ps[:, :w])

        # ---- initial x = ln(embed[idx]), normalized host-side ----
        nc.sync.dma_start(x_h[:], x0_d.ap().rearrange("(j p) d -> p j d",
                                                      p=P))
        tabs = [load_tab(0), load_tab(1)]
        enc_sb = load_enc(0)          # layer-0 enc prefetch
        nc.sync.dma_start(umask_sb[:], umask_d.ap())
        encv_sb = load_enc(1)
        dec_sb = load_dec()
        with tc.tile_pool(name="ps_init", bufs=2, space="PSUM") as ps_init:
            for j in range(NT):
                for k in range(KD):
                    transpose_into(xT_h[:, k, j * P:(j + 1) * P],
                                   x_h[:, j, k * P:(k + 1) * P], ps_init,
                                   nc.scalar if (j + k) % 2 else nc.vector)

        # ---- layers ----
        for layer in range(n_layers):
            # === Phase A (enc proj + relu) / rope / S pass-A, interleaved ===
            def emit_spassA(p, psSA_tiles):
                for ci, (r, base, w) in enumerate(PASSA):
                    nc.tensor.matmul(
                        psSA_tiles[ci][:, :w],
                        lhsT=qr8[:, :, p, r * P:(r + 1) * P],
                        rhs=qr8[:, :, p, base:base + w],
                        start=(p == 0), stop=(p == NPAIR - 1),
                        perf_mode=DR)

            a_scope = tc.tile_pool(name=f"psA_{layer}", bufs=2,
                                   space="PSUM")
            with tc.tile_pool(name=f"psSA_{layer}", bufs=1,
                              space="PSUM") as psSA:
                psSA_tiles = [psSA.tile([P, w], F32, name=f"sa{ci}",
                                        tag=f"sa{ci}")
                              for ci, (r, b, w) in enumerate(PASSA)]
                psA = a_scope.__enter__()
                for g in range(NG):
                    if g + 2 < NG:
                        tabs.append(load_tab(g + 2))
                    for mp in (2 * g, 2 * g + 1):
                        # enc proj for k-tiles (0, mp) and (1, mp)
                        for half in range(2):
                            m = half * NPAIR + mp
                            for c in range(2):
                                ps = psA.tile([P, 512], F32, name="psA")
                                for k in range(KD):
                                    nc.tensor.matmul(
                                        ps[:],
                                        lhsT=enc_sb[:, k, m * P:(m + 1) * P],
                                        rhs=xT_h[:, k,
                                                 c * 512:(c + 1) * 512],
                                        start=(k == 0), stop=(k == KD - 1))
                                dst = x_sp[:, half, mp,
                                           c * 512:(c + 1) * 512]
                                if g == 0 and (half + c) % 2 == 1:
                                    # DVE is idle before rope: split the
                                    # first group's relus to start rope ~2us
                                    # earlier
                                    nc.vector.tensor_scalar_max(
                                        out=dst, in0=ps[:], scalar1=0.0)
                                else:
                                    nc.scalar.activation(
                                        out=dst, in_=ps[:], func=RELU)
                    # rope group g: 2 pairs per DVE op, fp8 convert on Pool
                    ctt, stt = tabs[g]
                    xe = x_sp[:, 0, 2 * g:2 * g + 2, :]
                    xo = x_sp[:, 1, 2 * g:2 * g + 2, :]
                    # mul temps (w1/w2) are freed by the DVE itself (sub/add
                    # read them); the conv input lives in its own tag (w3) so
                    # a slow Pool convert never stalls the rope mul stream
                    t1 = wk16.tile([P, 2, T], HDT, name="w1")
                    t2 = wk16.tile([P, 2, T], HDT, name="w2", bufs=1)
                    nc.vector.tensor_mul(t1[:], xe, ctt[:])
                    nc.vector.tensor_mul(t2[:], xo, stt[:])
                    for h in range(2):
                        qe = wk16.tile([P, T], HDT, name="w3")
                        nc.vector.tensor_sub(qe[:], t1[:, h, :], t2[:, h, :])
                        nc.gpsimd.tensor_copy(out=qr8[:, 0, 2 * g + h, :],
                                              in_=qe[:])
                    t3 = wk16.tile([P, 2, T], HDT, name="w1")
                    t4 = wk16.tile([P, 2, T], HDT, name="w2", bufs=1)
                    nc.vector.tensor_mul(t3[:], xo, ctt[:])
                    nc.vector.tensor_mul(t4[:], xe, stt[:])
                    for h in range(2):
                        qo = wk16.tile([P, T], HDT, name="w3")
                        nc.vector.tensor_add(qo[:], t3[:, h, :], t4[:, h, :])
                        if g == NG - 1:
                            # last group: converts on Act (relus drained) so
                            # the S tail starts right at rope end
                            nc.scalar.copy(out=qr8[:, 1, 2 * g + h, :],
                                           in_=qo[:])
                        else:
                            nc.gpsimd.tensor_copy(
                                out=qr8[:, 1, 2 * g + h, :], in_=qo[:])
                    if g >= 1:
                        emit_spassA(2 * (g - 1), psSA_tiles)
                        emit_spassA(2 * (g - 1) + 1, psSA_tiles)
                del tabs[:NG]
                a_scope.__exit__(None, None, None)

                # psA's banks are free now (last relu done): accumulate row
                # 3 there while the rope tail finishes; pairs 0..13 are
                # ready so the PE idles less waiting for the last groups
                with tc.tile_pool(name=f"psB0_{layer}", bufs=1,
                                  space="PSUM") as psB0:
                    pb_tiles = [psB0.tile([P, w], F32, name=f"sb{ci}",
                                          tag=f"sb{ci}")
                                for ci, (r, b, w) in enumerate(PASSB0)]
                    for plo, phi in ((0, NPAIR - 2), (NPAIR - 2, NPAIR)):
                        for ci, (r, base, w) in enumerate(PASSB0):
                            for p in range(plo, phi):
                                nc.tensor.matmul(
                                    pb_tiles[ci][:, :w],
                                    lhsT=qr8[:, :, p, r * P:(r + 1) * P],
                                    rhs=qr8[:, :, p, base:base + w],
                                    start=(p == 0), stop=(p == NPAIR - 1),
                                    perf_mode=DR)
                    emit_spassA(NPAIR - 2, psSA_tiles)
                    emit_spassA(NPAIR - 1, psSA_tiles)
                    # copy S chunks to compact fp16 storage (+ diag mask),
                    # spread across Act/DVE
                    for ci, (r, base, w) in enumerate(PASSA):
                        s16_store(psSA_tiles[ci], r, base, w, eng=ci % 2)
                    for ci, (r, base, w) in enumerate(PASSB0):
                        s16_store(pb_tiles[ci], r, base, w, eng=ci % 2)

            # === S pass-B (rows 3..7, chunk-major) + ykv accumulation ===
            ykv_pre = stg.tile([P, NT, D], HDT, name="stg")
            ar_in = dram.tile([T, D], HDT, name=f"arin_{layer}",
                              tag=f"arin_{layer}")
            ar_in_v = ar_in.rearrange("(j p) d -> p j d", p=P)

            with tc.tile_pool(name=f"psSB_{layer}", bufs=3,
                              space="PSUM") as psSB, \
                 tc.tile_pool(name=f"psY_{layer}", bufs=2,
                              space="PSUM") as psY:
                def emit_ykv(j):
                    # diagonal block (i == j) last: its s16 row is the
                    # freshest, so earlier rows contract while it stores
                    ps = psY.tile([P, D], F32, name="psYt")
                    order = list(range(j)) + [j]
                    for n_, i in enumerate(order):
                        nc.tensor.matmul(
                            ps[:],
                            lhsT=s16[:, S_OFF[i] + (j - i) * P:
                                     S_OFF[i] + (j - i + 1) * P],
                            rhs=x_h[:, i, :],
                            start=(n_ == 0), stop=(n_ == j))
                    nc.scalar.mul(out=ykv_pre[:, j, :], in_=ps[:],
                                  mul=YKV_SCALE)

                # rows 0..3 are stored: their ykv groups + first AR half
                for j in range(4):
                    emit_ykv(j)
                nc.scalar.dma_start(ar_in_v[:, 0:4, :], ykv_pre[:, 0:4, :])
                for ci, (r, base, w) in enumerate(PASSB1):
                    ps = psSB.tile([P, w], F32, name="psSB")
                    for p in range(NPAIR):
                        nc.tensor.matmul(
                            ps[:],
                            lhsT=qr8[:, :, p, r * P:(r + 1) * P],
                            rhs=qr8[:, :, p, base:base + w],
                            start=(p == 0), stop=(p == NPAIR - 1),
                            perf_mode=DR)
                    s16_store(ps, r, base, w, eng=ci % 2)
                    emit_ykv(r)
                nc.scalar.dma_start(ar_in_v[:, 4:8, :], ykv_pre[:, 4:8, :])

            prefetch_next = layer + 1 < n_layers

            # === Phase C: pair AllReduce of ykv, layernorm, transpose ===
            ar_out = dram.tile([T, D], HDT, name=f"arout_{layer}",
                               tag=f"arout_{layer}")
            emit_allreduce(nc, PAIR_GROUPS, [ar_in.opt()], [ar_out.opt()])
            ykv_post = stg.tile([P, NT, D], HDT, name="stg")
            ar_out_v = ar_out.rearrange("(j p) d -> p j d", p=P)
            nc.sync.dma_start(ykv_post[:, 0:4, :], ar_out_v[:, 0:4, :])
            nc.sync.dma_start(ykv_post[:, 4:8, :], ar_out_v[:, 4:8, :])
            # next layer's tables + enc, gated past the AR window
            if prefetch_next:
                gate1 = statp.tile([P, 1], F32, name="gate")
                nc.vector.tensor_copy(out=gate1[:], in_=ykv_post[:, 7, 0:1])
                tabs = [load_tab(0, gate1), load_tab(1, gate1)]
                enc_next = load_enc(0, gate1)
            with tc.tile_pool(name=f"psT_{layer}", bufs=4,
                              space="PSUM") as psT:
                for lo in (0, 4):   # halves: D's c=0 needs only tiles 0..3
                    ln_batch(ykv_post, lambda j: ykv_post[:, j, :],
                             lo, lo + 4)   # in-place
                    for j in range(lo, lo + 4):
                        for k in range(KD):
                            transpose_into(ykvT_h[:, k, j * P:(j + 1) * P],
                                           ykv_post[:, j, k * P:(k + 1) * P],
                                           psT,
                                           nc.scalar if (j + k) % 2
                                           else nc.vector)

            # === Phase D: y_sp = relu(encv^T ykv^T); xy = x_sp*y_sp;
            # ymlp^T accumulated transposed (lhsT = decoder tile).
            # c-outer so the c=0 pass starts as soon as the first half of
            # ykvT's transposes land ===
            # Each c-half's accumulators complete at the end of its m-pass,
            # so the c=0 epilogue (PSUM copy, transpose to natural [T, D],
            # AllReduce staging) hides under the c=1 pass's compute.
            ymlp_nat = stg.tile([P, NT, D], HDT, name="stg")
            ar2_in = dram.tile([T, D], HDT, name=f"ar2in_{layer}",
                               tag=f"ar2in_{layer}")
            ar2_in_v = ar2_in.rearrange("(j p) d -> p j d", p=P)
            with tc.tile_pool(name=f"psD_{layer}", bufs=2,
                              space="PSUM") as psD, \
                 tc.tile_pool(name=f"psM_{layer}", bufs=1,
                              space="PSUM") as psM, \
                 tc.tile_pool(name=f"psTD_{layer}", bufs=2,
                              space="PSUM") as psTD:
                ymlpT_ps = [psM.tile([P, T], F32, name=f"ymlpT_ps{k}",
                                     tag=f"ymlpT_ps{k}") for k in range(KD)]

                def emit_dec(m, c, xy):
                    for k in range(KD):
                        nc.tensor.matmul(
                            ymlpT_ps[k][:, c * 512:(c + 1) * 512],
                            lhsT=dec_sb[:, m, k * P:(k + 1) * P],
                            rhs=xy[:],
                            start=(m == 0), stop=(m == NM - 1))

                def epilogue_c(c):
                    """PSUM->SBUF copies, transposes, AR staging for half c"""
                    ymk = [wk16.tile([P, 512], HDT, name="ymk")
                           for _ in range(KD)]
                    for k in range(KD):
                        src = ymlpT_ps[k][:, c * 512:(c + 1) * 512]
                        if (k + c) % 2:
                            nc.scalar.copy(out=ymk[k][:], in_=src)
                        else:
                            nc.vector.tensor_copy(out=ymk[k][:], in_=src)
                    for j in range(4 * c, 4 * c + 4):
                        jo = (j - 4 * c) * P
                        for k in range(KD):
                            transpose_into(ymlp_nat[:, j, k * P:(k + 1) * P],
                                           ymk[k][:, jo:jo + P], psTD,
                                           nc.scalar if (j + k) % 2
                                           else nc.vector)
                    nc.scalar.dma_start(ar2_in_v[:, 4 * c:4 * c + 4, :],
                                        ymlp_nat[:, 4 * c:4 * c + 4, :])

                for c in range(2):
                    pend = []   # deferred dec matmuls (lag 2 for pipelining)
                    for m in range(NM):
                        ps = psD.tile([P, 512], F32, name="psD")
                        for k in range(KD):
                            nc.tensor.matmul(
                                ps[:],
                                lhsT=encv_sb[:, k, m * P:(m + 1) * P],
                                rhs=ykvT_h[:, k, c * 512:(c + 1) * 512],
                                start=(k == 0), stop=(k == KD - 1))
                        ysp = wk16.tile([P, 512], HDT, name="w1")
                        nc.scalar.activation(out=ysp[:], in_=ps[:],
                                             func=RELU)
                        xy = wk16.tile([P, 512], HDT, name="w3")
                        nc.vector.tensor_mul(
                            xy[:], x_sp[:, m // NPAIR, m % NPAIR,
                                        c * 512:(c + 1) * 512], ysp[:])
                        pend.append((m, c, xy))
                        if len(pend) >= 3:
                            emit_dec(*pend.pop(0))
                        if c == 1 and m == 8:
                            epilogue_c(0)
                    for m, c_, xy in pend:
                        emit_dec(m, c_, xy)
                epilogue_c(1)

            # === Phase E: 8-way AllReduce of ymlp; x = ln(x + ln(ymlp)) ===
            ar2_out = dram.tile([T, D], HDT, name=f"ar2out_{layer}",
                                tag=f"ar2out_{layer}", addr_space="Shared")
            emit_allreduce(nc, ALL_GROUP, [ar2_in.opt()], [ar2_out.opt()])
            ymlp_post = stg.tile([P, NT, D], HDT, name="stg")
            ar2_out_v = ar2_out.rearrange("(j p) d -> p j d", p=P)
            nc.sync.dma_start(ymlp_post[:, 0:4, :], ar2_out_v[:, 0:4, :])
            nc.sync.dma_start(ymlp_post[:, 4:8, :], ar2_out_v[:, 4:8, :])
            # next layer's encv/dec, gated past the AR window
            if prefetch_next:
                gate2 = statp.tile([P, 1], F32, name="gate")
                nc.vector.tensor_copy(out=gate2[:], in_=ymlp_post[:, 7, 0:1])
                encv_next = load_enc(1, gate2)
                dec_next = load_dec(gate2)
            last = layer == n_layers - 1
            if last:
                lmh_sb = wk16.tile([P, KD, VOCAB], HDT, name="w2", bufs=1)
                for k in range(KD):
                    nc.sync.dma_start(lmh_sb[:, k, :],
                                      lmh_d.ap()[k * P:(k + 1) * P, :])
            with tc.tile_pool(name=f"psE_{layer}", bufs=4,
                              space="PSUM") as psE, \
                 tc.tile_pool(name=f"psL_{layer}", bufs=2,
                              space="PSUM") as psL:
                for lo in (0, 4):   # halves: next layer's A starts on 0..3
                    ln_batch(ymlp_post, lambda j: ymlp_post[:, j, :],
                             lo, lo + 4)   # in-place
                    for j in range(lo, lo + 4):
                        nc.vector.tensor_add(ymlp_post[:, j, :],
                                             ymlp_post[:, j, :],
                                             x_h[:, j, :])
                    ln_batch(ymlp_post, lambda j: x_h[:, j, :], lo, lo + 4)
                    for j in range(lo, lo + 4):
                        for k in range(KD):
                            transpose_into(xT_h[:, k, j * P:(j + 1) * P],
                                           x_h[:, j, k * P:(k + 1) * P],
                                           psE,
                                           nc.scalar if (j + k) % 2
                                           else nc.vector)
                        if last:
                            # logits = x @ lm_head, fused per t-tile
                            ps = psL.tile([P, VOCAB], F32, name="psLt")
                            for k in range(KD):
                                nc.tensor.matmul(
                                    ps[:],
                                    lhsT=xT_h[:, k, j * P:(j + 1) * P],
                                    rhs=lmh_sb[:, k, :],
                                    start=(k == 0), stop=(k == KD - 1))
                            lg = wk16.tile([P, VOCAB], F32, name="w1")
                            nc.scalar.copy(out=lg[:], in_=ps[:])
                            nc.sync.dma_start(
                                logits_d.ap()[j * P:(j + 1) * P, :], lg[:])

            if layer + 1 < n_layers:
                enc_sb, encv_sb, dec_sb = enc_next, encv_next, dec_next

        for _pool in (statp, stg, wk16, stp, ctp, decp,
                      encvp, encp, dram, persist):
            _pool.release()

    nc.compile()
    return nc


def _host_inputs(idx, embed, encoder, encoder_v, decoder, lm_head):
    """Build the 8 per-core input maps (host-side sharding)."""
    f16 = np.float16
    idx = np.asarray(idx).reshape(-1).astype(np.int64)
    embed = np.asarray(embed, np.float32)
    enc = np.asarray(encoder, np.float32)
    encv = np.asarray(encoder_v, np.float32)
    dec = np.asarray(decoder, np.float32)
    lmh = np.asarray(lm_head, np.float32)

    x0 = embed[idx]  # [T, D] gather on host (pure indexing)
    mu = x0.mean(-1, keepdims=True)
    var = x0.var(-1, keepdims=True)
    x0 = ((x0 - mu) / np.sqrt(var + EPS)).astype(np.float32)  # host LN

    # freqs exactly as the reference computes them (fp32)
    t = np.arange(0, N, dtype=np.float32)
    q = np.floor(t / 2.0) * 2.0
    freqs = (1.0 / ((2.0 ** 16) ** (q / N)) / TWO_PI).astype(np.float32)
    tvec = np.arange(T, dtype=np.float32)

    umask = (np.arange(P)[:, None] < np.arange(P)[None, :]).astype(np.float32)

    in_maps = []
    for d in range(N_CORES):
        h, half = d // 2, d % 2
        perm = np.concatenate([np.arange(0, NLOC, 2),
                               np.arange(1, NLOC, 2)]) + half * NLOC
        f_loc = freqs[perm[:NLOC // 2]]
        ph = (tvec[None, :] * f_loc[:, None]).astype(np.float32) % 1.0
        in_maps.append({
            "x0": np.ascontiguousarray(x0, f16),
            "encw": np.ascontiguousarray(enc[h][:, perm], f16),
            "encvw": np.ascontiguousarray(encv[h][:, perm], f16),
            "decw": np.ascontiguousarray(dec[h * N + perm, :], f16),
            "ct": np.ascontiguousarray(np.cos(TWO_PI * ph), f16),
            "st": np.ascontiguousarray(np.sin(TWO_PI * ph), f16),
            "lmh": np.ascontiguousarray(lmh, f16),
            "umask": umask,
        })
    return in_maps


def kernel(idx, embed, encoder, encoder_v, decoder, lm_head,
           _trace=False, _tmpdir=None):
    if "nc" not in _CACHE:
        _CACHE["nc"] = _build_program()
    nc = _CACHE["nc"]
    in_maps = _host_inputs(idx, embed, encoder, encoder_v, decoder, lm_head)
    res = bass_utils.run_bass_kernel_spmd(
        nc, in_maps, core_ids=list(range(N_CORES)),
        trace=_trace, tmpdir=_tmpdir)
    _CACHE["last_results"] = res
    logits = res.results[0]["logits"].astype(np.float32).reshape(B, T, VOCAB)
    return logits
